# revision 41
# baseline (speedup 1.0000x reference)
"""CPC (contrastive predictive coding) loss on 8 Trainium2 NeuronCores.

Problem: loss = mean over (t, k, i) of cross_entropy(scores[t,k,i,:], i) with
scores[t,k,i,j] = <c_proj[i,t], z[j,t+k]> / TEMP,  c_proj = c_seq @ W + b,
t in [0, Tm), k in [1, H], i,j in [0, B).

Distribution: sequence-parallel over anchor time t.  Every core runs an
identical program over TSLOT=14 anchor slots (7 "pair tiles" of 2 consecutive
anchors each); cores with fewer real anchors carry zero-padded slots whose
contributions are removed by per-core validity masks.  Each core returns a
(128,1) vector of partial sums; the host adds them up and divides by the term
count.

Math: per valid (t,k,i) group the loss term is
    lse - pos/T = max/T + ln(S) - pos/T,   S = sum_j exp((x_j - max)/T)
so the kernel accumulates three masked sums: sum vm*ln(S) (DVE stt after a
batched Ln), sum vm*max/T (tiny per-pair stt on the negated group maxima) and
-1/T * sum cm*x (cm = validity AND j==i "eye" mask, applied to the raw PSUM
scores tile) -- the positive logits come straight out of the big scores tile,
so no separate positive-pair matmuls are needed.

Per-core device pipeline (all matmuls bf16 inputs, fp32 accumulation):
  1. Plain contiguous DMA loads of host-pre-transposed z^T, c^T, W chunks.
  2. c_projT = (W-chunk as lhsT) @ c^T via PE into the shared PSUM pool; the
     per-partition bias is applied during the PSUM->SBUF bf16 copy, spread
     over ACT (Identity activation w/ bias) and DVE/Pool (stt w/ broadcast).
  3. Per pair tile (anchors t,t+1): one (128 x 31*64) PSUM scores tile via 16
     matmuls (4 col chunks x 4 contraction chunks).
  4. Softmax statistics, engine-balanced:
       - DVE: grouped reduce_max (negated), *1/T scale, tiny max-accum
       - Pool: dsb = ps - max broadcast for the first UF groups (bf16)
       - ACT: one Exp over dsb + X_FUSE per-group Exp instrs that read PSUM
         directly (per-partition bias = -max/T) and emit the group sum via
         accum_out, skipping those groups' subtract+reduce entirely
       - DVE: grouped sum of exp for the UF plain groups
       - DVE+Pool: the cm-masked raw-score accumulation (positive terms)
       - Ln batched once at the end (avoids ACT table ping-pong with Exp)
Masks (vm fp32, cm bf16) and bias are loaded once per launch outside the
benchmark loop; z/c/W stream in every iteration on two DMA queues with
double-buffered SBUF tiles so iteration i+1's loads overlap iteration i's
compute.
"""

import numpy as np
import ml_dtypes

B, T, D = 64, 128, 512
H = 30
TEMP = 0.07
NCORE = 8
TSLOT = 14            # padded anchor slots per core -> 7 pair tiles
NPAIR = TSLOT // 2
TS = TSLOT - 1 + H    # 43 z timesteps per core (slab + horizon halo)
G = H + 1             # 31 shift groups per pair tile
KCH = D // 128        # 4 contraction chunks
TM = T - H            # 98 real anchors

GA = 16               # groups in half-tile A (B gets G - GA = 15)
Y_ACC = 5             # B-half groups whose exp+sum run fused on ACT (accum)

_REAL = [13, 13, 12, 12, 12, 12, 12, 12]
_T0 = [0, 13, 26, 38, 50, 62, 74, 86]

_CACHE = {}


def _build_program(loop_n=None, variant="full", reps=1):
    import concourse.bass as bass
    import concourse.bacc as bacc
    import concourse.tile as tile
    import concourse.mybir as mybir
    from contextlib import ExitStack

    dt = mybir.dt
    AF = mybir.ActivationFunctionType
    ALU = mybir.AluOpType
    AX = mybir.AxisListType

    nc = bacc.Bacc("TRN2", debug=False, target_bir_lowering=False,
                   num_devices=NCORE)

    z_d = nc.dram_tensor("z_bf", [D, TS * B], dt.bfloat16, kind="ExternalInput").ap()
    c_d = nc.dram_tensor("c_bf", [D, B * TSLOT], dt.bfloat16, kind="ExternalInput").ap()
    w_d = nc.dram_tensor("w_bf", [D, D], dt.bfloat16, kind="ExternalInput").ap()
    b_d = nc.dram_tensor("b_f", [D], dt.float32, kind="ExternalInput").ap()
    vm_d = nc.dram_tensor("vm", [128, NPAIR * G], dt.float32, kind="ExternalInput").ap()
    cma_d = nc.dram_tensor("cma", [128, G * B], dt.bfloat16, kind="ExternalInput").ap()
    cmb_d = nc.dram_tensor("cmb", [128, G * B], dt.bfloat16, kind="ExternalInput").ap()
    out_d = nc.dram_tensor("partial", [128, 1], dt.float32, kind="ExternalOutput").ap()
    acc_d = None
    if variant == "dbg":
        acc_d = nc.dram_tensor("accdump", [128, 15], dt.float32,
                               kind="ExternalOutput").ap()
        sall_d = nc.dram_tensor("salldump", [128, NPAIR * G], dt.float32,
                                kind="ExternalOutput").ap()
        dsb_d = nc.dram_tensor("dsbdump", [128, GA * B], dt.bfloat16,
                               kind="ExternalOutput").ap()
        sba_d = nc.dram_tensor("sbadump", [128, GA * B], dt.bfloat16,
                               kind="ExternalOutput").ap()

    NROW = B * TSLOT          # 896 c rows
    GB = G * B                # 1984 columns of a pair tile
    NACC = 15                 # 0: lnS | 1..7: posA | 8..14: posB
    inv_t = 1.0 / TEMP

    with tile.TileContext(nc) as tc, ExitStack() as ctx:
        con = ctx.enter_context(tc.tile_pool(name="con", bufs=1))
        io = ctx.enter_context(tc.tile_pool(name="io", bufs=2))
        wrk = ctx.enter_context(tc.tile_pool(name="wrk", bufs=3))
        res = ctx.enter_context(tc.tile_pool(name="res", bufs=2))
        psm = ctx.enter_context(tc.tile_pool(name="psm", bufs=4, space="PSUM"))

        # ---------- per-launch constants (outside the bench loop) ----------
        b_sb = con.tile([128, KCH], dt.float32, tag="b", name="b_sb")
        nc.sync.dma_start(b_sb[:], b_d.rearrange("(c p) -> p c", p=128))
        vm_sb = con.tile([128, NPAIR * G], dt.float32, tag="vm", name="vm_sb")
        nc.sync.dma_start(vm_sb[:], vm_d)
        cma_sb = con.tile([128, GB], dt.bfloat16, tag="cma", name="cma_sb")
        nc.sync.dma_start(cma_sb[:], cma_d)
        cmb_sb = con.tile([128, GB], dt.bfloat16, tag="cmb", name="cmb_sb")
        nc.sync.dma_start(cmb_sb[:], cmb_d)

        def _body():
            # ---------------- streamed input loads ----------------
            # one DMA per tensor (all 4 contraction chunks side-by-side in
            # the free dim) -- per-DMA fixed overhead dominates on HW.
            # w + ct on the sync queue (they gate c_proj); zt on scalar.
            w_all = io.tile([128, KCH * D], dt.bfloat16, tag="w", name="w_all")
            nc.sync.dma_start(
                w_all[:].rearrange("p (k c) -> p k c", k=KCH),
                w_d.rearrange("(k p) c -> p k c", p=128))
            ct_all = io.tile([128, KCH * NROW], dt.bfloat16, tag="ct",
                             name="ct_all")
            nc.sync.dma_start(
                ct_all[:].rearrange("p (k c) -> p k c", k=KCH),
                c_d.rearrange("(k p) c -> p k c", p=128))
            zt_all = io.tile([128, KCH * B * TS], dt.bfloat16, tag="zt",
                             name="zt_all")
            nc.sync.dma_start(
                zt_all[:].rearrange("p (k c) -> p k c", k=KCH),
                z_d.rearrange("(k p) c -> p k c", p=128))
            w_sb = [w_all[:, k * D:(k + 1) * D] for k in range(KCH)]
            ct_sb = [ct_all[:, k * NROW:(k + 1) * NROW] for k in range(KCH)]
            zt_sb = [zt_all[:, k * B * TS:(k + 1) * B * TS] for k in range(KCH)]

            acc = res.tile([128, NACC], dt.float32, tag="acc", name="acc")
            nc.vector.memset(acc[:], 0.0)
            if variant == "dmaonly":
                for k in range(KCH):
                    nc.vector.tensor_reduce(acc[:, 0:1], zt_sb[k][:, 0:64],
                                            axis=AX.X, op=ALU.add)
                    nc.vector.tensor_reduce(acc[:, 1:2], ct_sb[k][:, 0:64],
                                            axis=AX.X, op=ALU.add)
                    nc.vector.tensor_reduce(acc[:, 2:3], w_sb[k][:, 0:64],
                                            axis=AX.X, op=ALU.add)
            s_all = res.tile([128, NPAIR * G], dt.float32, tag="sall", name="s_all")

            # ---------------- c_projT (bf16, (t, i) layout) ------------
            cq_sb = []
            for m in range(KCH if variant != "dmaonly" else 0):
                psc = psm.tile([128, GA * B], dt.float32, tag="ps", name="psc")
                for (n0, nn) in ((0, 512), (512, NROW - 512)):
                    for k in range(KCH):
                        nc.tensor.matmul(
                            psc[:, n0:n0 + nn],
                            w_sb[k][:, m * 128:(m + 1) * 128],
                            ct_sb[k][:, n0:n0 + nn],
                            start=(k == 0), stop=(k == KCH - 1),
                        )
                # cq memory layout is (t, i); psc arrives as (i, t), so the
                # write transposes: out is a plain (t,i) view of cq, in is
                # the (t,i)-ordered view of psc.
                cq = io.tile([128, NROW], dt.bfloat16, tag=f"cq{m}",
                             name=f"cq_sb{m}")
                cq_v = cq[:].rearrange("p (t i) -> p t i", i=B)
                psc_v = psc[:, 0:NROW].rearrange("p (i t) -> p t i", t=TSLOT)
                bias_v = b_sb[:, m:m + 1].broadcast_to((128, NROW)).rearrange(
                    "p (t i) -> p t i", i=B)
                # epilogue in two parallel halves (i 0..31 / 32..63) so the
                # PSUM slot frees in half the time (ACT + DVE; Pool cannot
                # read PSUM)
                half = B // 2
                nc.scalar.activation(cq_v[:, :, 0:half], psc_v[:, :, 0:half],
                                     AF.Identity, bias=b_sb[:, m:m + 1])
                nc.vector.scalar_tensor_tensor(
                    cq_v[:, :, half:B], psc_v[:, :, half:B], 1.0,
                    bias_v[:, :, half:B], op0=ALU.mult, op1=ALU.add)
                cq_sb.append(cq)

            # ------- 7 pair tiles, each as two half-tiles (A: groups
            # 0..GA-1, B: groups GA..30), software-pipelined ---------------
            # Only DVE and ACT can read PSUM, and Pool only supports plain
            # tensor_tensor ALU ops, so: each half is copied once to SBUF
            # (ACT, bf16) right after its matmuls -- this frees the 2-bank
            # PSUM slot almost immediately (PE keeps its max p-state) and
            # everything downstream runs from SBUF:
            #   ACT:  copyA, copyB -> expA, expB (+Y_ACC fused accum groups)
            #   Pool: subA, subB (tensor_tensor add with broadcast -max)
            #   DVE:  maxA, maxB; pos accums (telescoped, from dsb);
            #         exp-sums of pair p-1 (delayed so DVE never waits)
            GBB = G - GA          # groups in half B
            UB = GBB - Y_ACC      # B groups summed on DVE (rest via ACT accum)
            prev = None
            pend_d = []

            def stage_exp(st):
                # ACT: Exp halves + Y_ACC fused exp-sum groups (B tail)
                dsba, dsbb, cm3, s_t, p = st
                esba = wrk.tile([128, GA * B], dt.bfloat16, tag="esba",
                                name="esba")
                nc.scalar.activation(esba[:], dsba[:], AF.Exp, scale=inv_t)
                esbb = wrk.tile([128, UB * B], dt.bfloat16, tag="esbb",
                                name="esbb")
                nc.scalar.activation(esbb[:], dsbb[:, 0:UB * B], AF.Exp,
                                     scale=inv_t)
                for f in range(Y_ACC):
                    g = GA + UB + f
                    fe = wrk.tile([128, B], dt.bfloat16, tag=f"fe{f}",
                                  name=f"fe{f}")
                    nc.scalar.activation(
                        fe[:], dsbb[:, (UB + f) * B:(UB + f + 1) * B], AF.Exp,
                        scale=inv_t, accum_out=s_t[:, g:g + 1])
                return (esba, esbb, s_t)

            def stage_pos(st):
                # DVE: positive terms from dsb (the -max parts telescope
                # against the max-compensation since sum_j cm = vm)
                dsba, dsbb, cm3, s_t, p = st
                da3 = dsba[:].rearrange("p (g j) -> p g j", j=B)
                db3 = dsbb[:].rearrange("p (g j) -> p g j", j=B)
                junka = wrk.tile([128, GA * B], dt.bfloat16, tag="junka",
                                 name="junka")
                nc.vector.scalar_tensor_tensor(
                    junka[:].rearrange("p (g j) -> p g j", j=B),
                    da3, -inv_t, cm3[:, 0:GA, :],
                    op0=ALU.mult, op1=ALU.mult,
                    accum_out=acc[:, 1 + p:2 + p])
                junkb = wrk.tile([128, GBB * B], dt.bfloat16, tag="junkb",
                                 name="junkb")
                nc.vector.scalar_tensor_tensor(
                    junkb[:].rearrange("p (g j) -> p g j", j=B),
                    db3, -inv_t, cm3[:, GA:G, :],
                    op0=ALU.mult, op1=ALU.mult,
                    accum_out=acc[:, 8 + p:9 + p])

            def stage_d(st):
                # DVE: grouped exp-sums
                pesba, pesbb, pst = st
                nc.vector.tensor_reduce(
                    pst[:, 0:GA],
                    pesba[:].rearrange("p (g j) -> p g j", j=B),
                    axis=AX.X, op=ALU.add)
                nc.vector.tensor_reduce(
                    pst[:, GA:GA + UB],
                    pesbb[:].rearrange("p (g j) -> p g j", j=B),
                    axis=AX.X, op=ALU.add)
            for p in range(NPAIR if variant != "dmaonly" else 0):
                psa = psm.tile([128, GA * B], dt.float32, tag="ps", name="psa")
                for (g0, gn) in ((0, 8), (8, GA - 8)):
                    for k in range(KCH):
                        nc.tensor.matmul(
                            psa[:, g0 * B:(g0 + gn) * B],
                            cq_sb[k][:, 2 * p * B:(2 * p + 2) * B],
                            zt_sb[k][:, (2 * p + g0) * B:(2 * p + g0 + gn) * B],
                            start=(k == 0), stop=(k == KCH - 1),
                        )
                psb = psm.tile([128, GA * B], dt.float32, tag="ps", name="psb")
                for (g0, gn) in ((GA, 8), (GA + 8, GBB - 8)):
                    for k in range(KCH):
                        nc.tensor.matmul(
                            psb[:, (g0 - GA) * B:(g0 - GA + gn) * B],
                            cq_sb[k][:, 2 * p * B:(2 * p + 2) * B],
                            zt_sb[k][:, (2 * p + g0) * B:(2 * p + g0 + gn) * B],
                            start=(k == 0), stop=(k == KCH - 1),
                        )

                if variant == "noce":
                    junkc = wrk.tile([128, 1], dt.float32, tag="junkc",
                                     name="junkc")
                    nc.vector.tensor_reduce(junkc[:], psa[:, 0:B],
                                            axis=AX.X, op=ALU.add)
                    nc.vector.tensor_reduce(junkc[:], psb[:, 0:B],
                                            axis=AX.X, op=ALU.add)
                    continue

                cm_sb = cmb_sb if p == NPAIR - 1 else cma_sb
                cm3 = cm_sb[:].rearrange("p (g j) -> p g j", j=B)
                s_t = s_all[:, p * G:(p + 1) * G]

                # ACT: one bf16 copy per half frees the PSUM slot.  These
                # lead ACT's (strictly in-order) stream each pair; the exps
                # of the previous pair are emitted after them so a copy is
                # never queued behind work that waits on Pool/DVE.
                sba = wrk.tile([128, GA * B], dt.bfloat16, tag="sba", name="sba")
                nc.scalar.activation(sba[:], psa[:], AF.Copy)
                sbb = wrk.tile([128, GBB * B], dt.bfloat16, tag="sbb", name="sbb")
                nc.scalar.activation(sbb[:], psb[:, 0:GBB * B], AF.Copy)
                sa3 = sba[:].rearrange("p (g j) -> p g j", j=B)
                sb3 = sbb[:].rearrange("p (g j) -> p g j", j=B)

                # DVE: grouped max halves (negated, separate tiles)
                nma = wrk.tile([128, GA], dt.float32, tag="nma", name="nma")
                nc.vector.tensor_reduce(nma[:], sa3, axis=AX.X,
                                        op=ALU.max, negate=True)
                nmb = wrk.tile([128, GBB], dt.float32, tag="nmb", name="nmb")
                nc.vector.tensor_reduce(nmb[:], sb3, axis=AX.X,
                                        op=ALU.max, negate=True)

                # Pool: dsb = sb + (-max) broadcast (bf16), per half
                dsba = wrk.tile([128, GA * B], dt.bfloat16, tag="dsba",
                                name="dsba")
                nc.gpsimd.tensor_tensor(
                    dsba[:].rearrange("p (g j) -> p g j", j=B),
                    sa3, nma[:].broadcast_to((128, GA, B)), op=ALU.add)
                dsbb = wrk.tile([128, GBB * B], dt.bfloat16, tag="dsbb",
                                name="dsbb")
                nc.gpsimd.tensor_tensor(
                    dsbb[:].rearrange("p (g j) -> p g j", j=B),
                    sb3, nmb[:].broadcast_to((128, GBB, B)), op=ALU.add)

                if variant == "dbg" and p == 0:
                    nc.gpsimd.dma_start(dsb_d, dsba[:])
                    nc.gpsimd.dma_start(sba_d, sba[:])

                # software-pipelined tails: exps + pos for pair p-1,
                # exp-sums for pair p-2
                if prev is not None:
                    pend_d.append(stage_exp(prev))
                    stage_pos(prev)
                if len(pend_d) > 1:
                    stage_d(pend_d.pop(0))
                prev = (dsba, dsbb, cm3, s_t, p)
            if variant in ("full", "dbg") and prev is not None:
                pend_d.append(stage_exp(prev))
                stage_pos(prev)
                for st in pend_d:
                    stage_d(st)

            if variant == "dbg":
                nc.gpsimd.dma_start(acc_d, acc[:])
                nc.gpsimd.dma_start(sall_d, s_all[:])
            if variant not in ("full", "dbg"):
                part0 = res.tile([128, 1], dt.float32, tag="part", name="part0")
                nc.vector.tensor_reduce(part0[:], acc[:], axis=AX.X, op=ALU.add)
                nc.gpsimd.dma_start(out_d, part0[:])
                return
            logs_all = res.tile([128, NPAIR * G], dt.float32, tag="logsall",
                                name="logs_all")
            nc.scalar.activation(logs_all[:], s_all[:], AF.Ln)
            junkl = res.tile([128, NPAIR * G], dt.float32, tag="junkl", name="junkl")
            nc.vector.scalar_tensor_tensor(
                junkl[:], logs_all[:], 1.0, vm_sb[:], op0=ALU.mult, op1=ALU.mult,
                accum_out=acc[:, 0:1])
            part = res.tile([128, 1], dt.float32, tag="part", name="part")
            nc.vector.tensor_reduce(part[:], acc[:], axis=AX.X, op=ALU.add)
            nc.gpsimd.dma_start(out_d, part[:])

        if loop_n:
            with tc.For_i(0, loop_n, 1):
                for _ in range(reps):
                    _body()
        else:
            for _ in range(reps):
                _body()

    nc.compile()
    return nc


def get_program(loop_n=None, variant="full", reps=1):
    key = ("nc", loop_n, variant, reps)
    if key not in _CACHE:
        _CACHE[key] = _build_program(loop_n, variant, reps)
    return _CACHE[key]


def make_core_inputs(m, z, c, W, b):
    """Host-side sharding + bf16 cast for core m."""
    bf = ml_dtypes.bfloat16
    t0, nreal = _T0[m], _REAL[m]

    # device-side layouts: zT (D, (s, i)), cT (D, (i, t)) -- transposed on
    # the host so the device does plain contiguous DMA loads (no xbar)
    s_lo = t0 + 1
    n_avail = min(TS, T - s_lo)
    zslab = np.zeros((D, TS, B), dtype=bf)
    zslab[:, :n_avail] = z[:, s_lo:s_lo + n_avail].astype(bf).transpose(2, 1, 0)
    zslab = zslab.reshape(D, TS * B)

    cslab = np.zeros((D, B, TSLOT), dtype=bf)
    cslab[:, :, :nreal] = c[:, t0:t0 + nreal].astype(bf).transpose(2, 0, 1)
    cslab = cslab.reshape(D, B * TSLOT)

    # pair-tile validity: partition p = half*64 + i, half anchored at t+half
    p_idx = np.arange(128)
    g_idx = np.arange(G)
    th = p_idx[:, None, None] // B                     # (128,1,1)
    pp = np.arange(NPAIR)[None, :, None]               # (1,7,1)
    gg = g_idx[None, None, :]                          # (1,1,31)
    slot = 2 * pp + th
    gvalid = np.where(th == 0, gg <= H - 1, (gg >= 1) & (gg <= H))
    vm = ((slot < nreal) & gvalid).astype(np.float32).reshape(128, NPAIR * G)

    # (e) masks: cm[p, g*64+j] = gvalid * (j == p%64); cmb adds the last
    # pair's slot validity (pairs 0..5 are always fully populated)
    eye = (np.arange(B)[None, :] == (p_idx % B)[:, None])          # (128,64)
    gv2 = np.where((p_idx // B)[:, None] == 0,
                   g_idx[None, :] <= H - 1,
                   (g_idx[None, :] >= 1) & (g_idx[None, :] <= H))  # (128,31)
    cma = (gv2[:, :, None] & eye[:, None, :]).astype(bf).reshape(128, G * B)
    slot6 = 12 + (p_idx // B)                                      # (128,)
    cmb = ((gv2 & (slot6 < nreal)[:, None])[:, :, None]
           & eye[:, None, :]).astype(bf).reshape(128, G * B)

    return {
        "z_bf": zslab,
        "c_bf": cslab,
        "w_bf": W.astype(bf),
        "b_f": b.astype(np.float32),
        "vm": vm,
        "cma": cma,
        "cmb": cmb,
    }


def kernel(z_seq, c_seq, W_cpc, b_cpc):
    z = np.asarray(z_seq, dtype=np.float32)
    c = np.asarray(c_seq, dtype=np.float32)
    W = np.asarray(W_cpc, dtype=np.float32)
    b = np.asarray(b_cpc, dtype=np.float32)

    nc = get_program()
    in_maps = [make_core_inputs(m, z, c, W, b) for m in range(NCORE)]

    from concourse.bass_utils import run_bass_kernel_spmd
    res = run_bass_kernel_spmd(nc, in_maps, core_ids=list(range(NCORE)))

    tot = sum(float(r["partial"].astype(np.float64).sum()) for r in res.results)
    return np.float32(tot / (TM * H * B))


if __name__ == "__main__":
    rng = np.random.default_rng(0)
    out = kernel(
        rng.standard_normal((B, T, D), dtype=np.float32),
        rng.standard_normal((B, T, D), dtype=np.float32),
        (rng.standard_normal((D, D)) / np.sqrt(D)).astype(np.float32),
        (rng.standard_normal(D) * 0.01).astype(np.float32),
    )
    print("loss:", out)


# revision 43
# speedup vs baseline: 1.4385x; 1.4385x over previous
"""CPC (contrastive predictive coding) loss on 8 Trainium2 NeuronCores.

Problem: loss = mean over (t, k, i) of cross_entropy(scores[t,k,i,:], i) with
scores[t,k,i,j] = <c_proj[i,t], z[j,t+k]> / TEMP,  c_proj = c_seq @ W + b,
t in [0, Tm), k in [1, H], i,j in [0, B).

Distribution: sequence-parallel over anchor time t.  Every core runs an
identical program over TSLOT=14 anchor slots (7 "pair tiles" of 2 consecutive
anchors each); cores with fewer real anchors carry zero-padded slots whose
contributions are removed by per-core validity masks.  Each core returns a
(128,1) vector of partial sums; the host adds them up and divides by the term
count.

Math: per valid (t,k,i) group the loss term is
    lse - pos/T = max/T + ln(S) - pos/T,   S = sum_j exp((x_j - max)/T)
so the kernel accumulates three masked sums: sum vm*ln(S) (DVE stt after a
batched Ln), sum vm*max/T (tiny per-pair stt on the negated group maxima) and
-1/T * sum cm*x (cm = validity AND j==i "eye" mask, applied to the raw PSUM
scores tile) -- the positive logits come straight out of the big scores tile,
so no separate positive-pair matmuls are needed.

Per-core device pipeline (all matmuls bf16 inputs, fp32 accumulation):
  1. One merged DMA per input tensor (all 4 contraction chunks in one
     transfer -- per-DMA fixed overhead dominates on this runtime).
  2. c_projT = (W-chunk as lhsT) @ c^T via PE into the shared PSUM pool;
     bias applied during the transposing PSUM->SBUF bf16 epilogue (ACT+DVE).
  3. Per pair tile (anchors t,t+1): scores as two 2-bank PSUM half-tiles
     (16/15 shift groups), 8 matmuls each; four half-tiles in flight.
  4. Softmax statistics, engine-constrained (only DVE/ACT read PSUM; Pool
     only runs plain tensor_tensor ALU ops):
       - ACT: one bf16 Copy per half frees the PSUM slot immediately
       - DVE: grouped reduce_max (negated) per half, from the SBUF copy
       - Pool: dsb = sb + (-max) broadcast (tensor_tensor add, bf16)
       - ACT: Exp halves + Y_ACC per-group Exp w/ accum_out (group sums)
       - DVE: grouped exp-sums; cm-masked dsb accumulation whose -max part
         telescopes away the max-compensation term (sum_j cm = vm)
       - Ln batched once at the end (avoids ACT table ping-pong with Exp)
  Every cross-engine stage is software-pipelined one pair behind its
  producer (exp-sums two pairs) so no strictly in-order engine queue --
  ACT especially -- ever head-of-line-blocks on another engine's future
  value.  Masks and bias load once per launch outside the bench loop.
"""

import numpy as np
import ml_dtypes

B, T, D = 64, 128, 512
H = 30
TEMP = 0.07
NCORE = 8
TSLOT = 14            # padded anchor slots per core -> 7 pair tiles
NPAIR = TSLOT // 2
TS = TSLOT - 1 + H    # 43 z timesteps per core (slab + horizon halo)
G = H + 1             # 31 shift groups per pair tile
KCH = D // 128        # 4 contraction chunks
TM = T - H            # 98 real anchors

GA = 16               # groups in half-tile A (B gets G - GA = 15)
Y_ACC = 5             # B-half groups whose exp+sum run fused on ACT (accum)

_REAL = [13, 13, 12, 12, 12, 12, 12, 12]
_T0 = [0, 13, 26, 38, 50, 62, 74, 86]

_CACHE = {}


def _build_program(loop_n=None, variant="full", reps=1):
    import concourse.bass as bass
    import concourse.bacc as bacc
    import concourse.tile as tile
    import concourse.mybir as mybir
    from contextlib import ExitStack

    dt = mybir.dt
    AF = mybir.ActivationFunctionType
    ALU = mybir.AluOpType
    AX = mybir.AxisListType

    nc = bacc.Bacc("TRN2", debug=False, target_bir_lowering=False,
                   num_devices=NCORE)

    z_d = nc.dram_tensor("z_bf", [D, TS * B], dt.bfloat16, kind="ExternalInput").ap()
    c_d = nc.dram_tensor("c_bf", [D, B * TSLOT], dt.bfloat16, kind="ExternalInput").ap()
    w_d = nc.dram_tensor("w_bf", [D, D], dt.bfloat16, kind="ExternalInput").ap()
    b_d = nc.dram_tensor("b_f", [D], dt.float32, kind="ExternalInput").ap()
    vm_d = nc.dram_tensor("vm", [128, NPAIR * G], dt.float32, kind="ExternalInput").ap()
    cma_d = nc.dram_tensor("cma", [128, G * B], dt.bfloat16, kind="ExternalInput").ap()
    cmb_d = nc.dram_tensor("cmb", [128, G * B], dt.bfloat16, kind="ExternalInput").ap()
    out_d = nc.dram_tensor("partial", [128, 1], dt.float32, kind="ExternalOutput").ap()
    acc_d = None
    if variant == "dbg":
        acc_d = nc.dram_tensor("accdump", [128, 15], dt.float32,
                               kind="ExternalOutput").ap()
        sall_d = nc.dram_tensor("salldump", [128, NPAIR * G], dt.float32,
                                kind="ExternalOutput").ap()
        dsb_d = nc.dram_tensor("dsbdump", [128, GA * B], dt.bfloat16,
                               kind="ExternalOutput").ap()
        sba_d = nc.dram_tensor("sbadump", [128, GA * B], dt.bfloat16,
                               kind="ExternalOutput").ap()

    NROW = B * TSLOT          # 896 c rows
    GB = G * B                # 1984 columns of a pair tile
    NACC = 15                 # 0: lnS | 1..7: posA | 8..14: posB
    inv_t = 1.0 / TEMP

    with tile.TileContext(nc) as tc, ExitStack() as ctx:
        con = ctx.enter_context(tc.tile_pool(name="con", bufs=1))
        io = ctx.enter_context(tc.tile_pool(name="io", bufs=2))
        wrk = ctx.enter_context(tc.tile_pool(name="wrk", bufs=3))
        res = ctx.enter_context(tc.tile_pool(name="res", bufs=2))
        psm = ctx.enter_context(tc.tile_pool(name="psm", bufs=4, space="PSUM"))

        # ---------- per-launch constants (outside the bench loop) ----------
        b_sb = con.tile([128, KCH], dt.float32, tag="b", name="b_sb")
        nc.sync.dma_start(b_sb[:], b_d.rearrange("(c p) -> p c", p=128))
        vm_sb = con.tile([128, NPAIR * G], dt.float32, tag="vm", name="vm_sb")
        nc.sync.dma_start(vm_sb[:], vm_d)
        cma_sb = con.tile([128, GB], dt.bfloat16, tag="cma", name="cma_sb")
        nc.sync.dma_start(cma_sb[:], cma_d)
        cmb_sb = con.tile([128, GB], dt.bfloat16, tag="cmb", name="cmb_sb")
        nc.sync.dma_start(cmb_sb[:], cmb_d)

        def _body():
            # ---------------- streamed input loads ----------------
            # one DMA per tensor (all 4 contraction chunks side-by-side in
            # the free dim) -- per-DMA fixed overhead dominates on HW.
            # w + ct on the sync queue (they gate c_proj); zt on scalar.
            w_all = io.tile([128, KCH * D], dt.bfloat16, tag="w", name="w_all")
            nc.sync.dma_start(
                w_all[:].rearrange("p (k c) -> p k c", k=KCH),
                w_d.rearrange("(k p) c -> p k c", p=128))
            ct_all = io.tile([128, KCH * NROW], dt.bfloat16, tag="ct",
                             name="ct_all")
            nc.sync.dma_start(
                ct_all[:].rearrange("p (k c) -> p k c", k=KCH),
                c_d.rearrange("(k p) c -> p k c", p=128))
            zt_all = io.tile([128, KCH * B * TS], dt.bfloat16, tag="zt",
                             name="zt_all")
            nc.sync.dma_start(
                zt_all[:].rearrange("p (k c) -> p k c", k=KCH),
                z_d.rearrange("(k p) c -> p k c", p=128))
            w_sb = [w_all[:, k * D:(k + 1) * D] for k in range(KCH)]
            ct_sb = [ct_all[:, k * NROW:(k + 1) * NROW] for k in range(KCH)]
            zt_sb = [zt_all[:, k * B * TS:(k + 1) * B * TS] for k in range(KCH)]

            acc = res.tile([128, NACC], dt.float32, tag="acc", name="acc")
            nc.vector.memset(acc[:], 0.0)
            if variant == "dmaonly":
                for k in range(KCH):
                    nc.vector.tensor_reduce(acc[:, 0:1], zt_sb[k][:, 0:64],
                                            axis=AX.X, op=ALU.add)
                    nc.vector.tensor_reduce(acc[:, 1:2], ct_sb[k][:, 0:64],
                                            axis=AX.X, op=ALU.add)
                    nc.vector.tensor_reduce(acc[:, 2:3], w_sb[k][:, 0:64],
                                            axis=AX.X, op=ALU.add)
            s_all = res.tile([128, NPAIR * G], dt.float32, tag="sall", name="s_all")

            # ---------------- c_projT (bf16, (t, i) layout) ------------
            cq_sb = []
            for m in range(KCH if variant != "dmaonly" else 0):
                psc = psm.tile([128, GA * B], dt.float32, tag="ps", name="psc")
                for (n0, nn) in ((0, 512), (512, NROW - 512)):
                    for k in range(KCH):
                        nc.tensor.matmul(
                            psc[:, n0:n0 + nn],
                            w_sb[k][:, m * 128:(m + 1) * 128],
                            ct_sb[k][:, n0:n0 + nn],
                            start=(k == 0), stop=(k == KCH - 1),
                        )
                # cq memory layout is (t, i); psc arrives as (i, t), so the
                # write transposes: out is a plain (t,i) view of cq, in is
                # the (t,i)-ordered view of psc.
                cq = io.tile([128, NROW], dt.bfloat16, tag=f"cq{m}",
                             name=f"cq_sb{m}")
                cq_v = cq[:].rearrange("p (t i) -> p t i", i=B)
                psc_v = psc[:, 0:NROW].rearrange("p (i t) -> p t i", t=TSLOT)
                bias_v = b_sb[:, m:m + 1].broadcast_to((128, NROW)).rearrange(
                    "p (t i) -> p t i", i=B)
                # epilogue in two parallel halves (i 0..31 / 32..63) so the
                # PSUM slot frees in half the time (ACT + DVE; Pool cannot
                # read PSUM)
                half = B // 2
                nc.scalar.activation(cq_v[:, :, 0:half], psc_v[:, :, 0:half],
                                     AF.Identity, bias=b_sb[:, m:m + 1])
                nc.vector.scalar_tensor_tensor(
                    cq_v[:, :, half:B], psc_v[:, :, half:B], 1.0,
                    bias_v[:, :, half:B], op0=ALU.mult, op1=ALU.add)
                cq_sb.append(cq)

            # ------- 7 pair tiles, each as two half-tiles (A: groups
            # 0..GA-1, B: groups GA..30), software-pipelined ---------------
            # Only DVE and ACT can read PSUM, and Pool only supports plain
            # tensor_tensor ALU ops, so: each half is copied once to SBUF
            # (ACT, bf16) right after its matmuls -- this frees the 2-bank
            # PSUM slot almost immediately (PE keeps its max p-state) and
            # everything downstream runs from SBUF:
            #   ACT:  copyA, copyB -> expA, expB (+Y_ACC fused accum groups)
            #   Pool: subA, subB (tensor_tensor add with broadcast -max)
            #   DVE:  maxA, maxB; pos accums (telescoped, from dsb);
            #         exp-sums of pair p-1 (delayed so DVE never waits)
            GBB = G - GA          # groups in half B
            UB = GBB - Y_ACC      # B groups summed on DVE (rest via ACT accum)
            prev = None
            pend_d = []

            def stage_exp(st):
                # ACT: Exp halves + Y_ACC fused exp-sum groups (B tail)
                dsba, dsbb, cm3, s_t, p = st
                esba = wrk.tile([128, GA * B], dt.bfloat16, tag="esba",
                                name="esba")
                nc.scalar.activation(esba[:], dsba[:], AF.Exp, scale=inv_t)
                esbb = wrk.tile([128, UB * B], dt.bfloat16, tag="esbb",
                                name="esbb")
                nc.scalar.activation(esbb[:], dsbb[:, 0:UB * B], AF.Exp,
                                     scale=inv_t)
                for f in range(Y_ACC):
                    g = GA + UB + f
                    fe = wrk.tile([128, B], dt.bfloat16, tag=f"fe{f}",
                                  name=f"fe{f}")
                    nc.scalar.activation(
                        fe[:], dsbb[:, (UB + f) * B:(UB + f + 1) * B], AF.Exp,
                        scale=inv_t, accum_out=s_t[:, g:g + 1])
                return (esba, esbb, s_t)

            def stage_pos(st):
                # DVE: positive terms from dsb (the -max parts telescope
                # against the max-compensation since sum_j cm = vm)
                dsba, dsbb, cm3, s_t, p = st
                da3 = dsba[:].rearrange("p (g j) -> p g j", j=B)
                db3 = dsbb[:].rearrange("p (g j) -> p g j", j=B)
                junka = wrk.tile([128, GA * B], dt.bfloat16, tag="junka",
                                 name="junka")
                nc.vector.scalar_tensor_tensor(
                    junka[:].rearrange("p (g j) -> p g j", j=B),
                    da3, -inv_t, cm3[:, 0:GA, :],
                    op0=ALU.mult, op1=ALU.mult,
                    accum_out=acc[:, 1 + p:2 + p])
                junkb = wrk.tile([128, GBB * B], dt.bfloat16, tag="junkb",
                                 name="junkb")
                nc.vector.scalar_tensor_tensor(
                    junkb[:].rearrange("p (g j) -> p g j", j=B),
                    db3, -inv_t, cm3[:, GA:G, :],
                    op0=ALU.mult, op1=ALU.mult,
                    accum_out=acc[:, 8 + p:9 + p])

            def stage_d(st):
                # DVE: grouped exp-sums
                pesba, pesbb, pst = st
                nc.vector.tensor_reduce(
                    pst[:, 0:GA],
                    pesba[:].rearrange("p (g j) -> p g j", j=B),
                    axis=AX.X, op=ALU.add)
                nc.vector.tensor_reduce(
                    pst[:, GA:GA + UB],
                    pesbb[:].rearrange("p (g j) -> p g j", j=B),
                    axis=AX.X, op=ALU.add)
            for p in range(NPAIR if variant != "dmaonly" else 0):
                psa = psm.tile([128, GA * B], dt.float32, tag="ps", name="psa")
                for (g0, gn) in ((0, 8), (8, GA - 8)):
                    for k in range(KCH):
                        nc.tensor.matmul(
                            psa[:, g0 * B:(g0 + gn) * B],
                            cq_sb[k][:, 2 * p * B:(2 * p + 2) * B],
                            zt_sb[k][:, (2 * p + g0) * B:(2 * p + g0 + gn) * B],
                            start=(k == 0), stop=(k == KCH - 1),
                        )
                psb = psm.tile([128, GA * B], dt.float32, tag="ps", name="psb")
                for (g0, gn) in ((GA, 8), (GA + 8, GBB - 8)):
                    for k in range(KCH):
                        nc.tensor.matmul(
                            psb[:, (g0 - GA) * B:(g0 - GA + gn) * B],
                            cq_sb[k][:, 2 * p * B:(2 * p + 2) * B],
                            zt_sb[k][:, (2 * p + g0) * B:(2 * p + g0 + gn) * B],
                            start=(k == 0), stop=(k == KCH - 1),
                        )

                if variant == "noce":
                    junkc = wrk.tile([128, 1], dt.float32, tag="junkc",
                                     name="junkc")
                    nc.vector.tensor_reduce(junkc[:], psa[:, 0:B],
                                            axis=AX.X, op=ALU.add)
                    nc.vector.tensor_reduce(junkc[:], psb[:, 0:B],
                                            axis=AX.X, op=ALU.add)
                    continue

                cm_sb = cmb_sb if p == NPAIR - 1 else cma_sb
                cm3 = cm_sb[:].rearrange("p (g j) -> p g j", j=B)
                s_t = s_all[:, p * G:(p + 1) * G]

                # ACT: one bf16 copy per half frees the PSUM slot.  These
                # lead ACT's (strictly in-order) stream each pair; the exps
                # of the previous pair are emitted after them so a copy is
                # never queued behind work that waits on Pool/DVE.
                sba = wrk.tile([128, GA * B], dt.bfloat16, tag="sba", name="sba")
                nc.scalar.activation(sba[:], psa[:], AF.Copy)
                sbb = wrk.tile([128, GBB * B], dt.bfloat16, tag="sbb", name="sbb")
                nc.scalar.activation(sbb[:], psb[:, 0:GBB * B], AF.Copy)
                sa3 = sba[:].rearrange("p (g j) -> p g j", j=B)
                sb3 = sbb[:].rearrange("p (g j) -> p g j", j=B)

                # DVE: grouped max halves (negated, separate tiles)
                nma = wrk.tile([128, GA], dt.float32, tag="nma", name="nma")
                nc.vector.tensor_reduce(nma[:], sa3, axis=AX.X,
                                        op=ALU.max, negate=True)
                nmb = wrk.tile([128, GBB], dt.float32, tag="nmb", name="nmb")
                nc.vector.tensor_reduce(nmb[:], sb3, axis=AX.X,
                                        op=ALU.max, negate=True)

                # Pool: dsb = sb + (-max) broadcast (bf16), per half
                dsba = wrk.tile([128, GA * B], dt.bfloat16, tag="dsba",
                                name="dsba")
                nc.gpsimd.tensor_tensor(
                    dsba[:].rearrange("p (g j) -> p g j", j=B),
                    sa3, nma[:].broadcast_to((128, GA, B)), op=ALU.add)
                dsbb = wrk.tile([128, GBB * B], dt.bfloat16, tag="dsbb",
                                name="dsbb")
                nc.gpsimd.tensor_tensor(
                    dsbb[:].rearrange("p (g j) -> p g j", j=B),
                    sb3, nmb[:].broadcast_to((128, GBB, B)), op=ALU.add)

                if variant == "dbg" and p == 0:
                    nc.gpsimd.dma_start(dsb_d, dsba[:])
                    nc.gpsimd.dma_start(sba_d, sba[:])

                # software-pipelined tails: exps + pos for pair p-1,
                # exp-sums for pair p-2
                if prev is not None:
                    pend_d.append(stage_exp(prev))
                    stage_pos(prev)
                if len(pend_d) > 1:
                    stage_d(pend_d.pop(0))
                prev = (dsba, dsbb, cm3, s_t, p)
            if variant in ("full", "dbg") and prev is not None:
                pend_d.append(stage_exp(prev))
                stage_pos(prev)
                for st in pend_d:
                    stage_d(st)

            if variant == "dbg":
                nc.gpsimd.dma_start(acc_d, acc[:])
                nc.gpsimd.dma_start(sall_d, s_all[:])
            if variant not in ("full", "dbg"):
                part0 = res.tile([128, 1], dt.float32, tag="part", name="part0")
                nc.vector.tensor_reduce(part0[:], acc[:], axis=AX.X, op=ALU.add)
                nc.gpsimd.dma_start(out_d, part0[:])
                return
            logs_all = res.tile([128, NPAIR * G], dt.float32, tag="logsall",
                                name="logs_all")
            nc.scalar.activation(logs_all[:], s_all[:], AF.Ln)
            junkl = res.tile([128, NPAIR * G], dt.float32, tag="junkl", name="junkl")
            nc.vector.scalar_tensor_tensor(
                junkl[:], logs_all[:], 1.0, vm_sb[:], op0=ALU.mult, op1=ALU.mult,
                accum_out=acc[:, 0:1])
            part = res.tile([128, 1], dt.float32, tag="part", name="part")
            nc.vector.tensor_reduce(part[:], acc[:], axis=AX.X, op=ALU.add)
            nc.gpsimd.dma_start(out_d, part[:])

        if loop_n:
            with tc.For_i(0, loop_n, 1):
                for _ in range(reps):
                    _body()
        else:
            for _ in range(reps):
                _body()

    nc.compile()
    return nc


def get_program(loop_n=None, variant="full", reps=1):
    key = ("nc", loop_n, variant, reps)
    if key not in _CACHE:
        _CACHE[key] = _build_program(loop_n, variant, reps)
    return _CACHE[key]


def make_core_inputs(m, z, c, W, b):
    """Host-side sharding + bf16 cast for core m."""
    bf = ml_dtypes.bfloat16
    t0, nreal = _T0[m], _REAL[m]

    # device-side layouts: zT (D, (s, i)), cT (D, (i, t)) -- transposed on
    # the host so the device does plain contiguous DMA loads (no xbar)
    s_lo = t0 + 1
    n_avail = min(TS, T - s_lo)
    zslab = np.zeros((D, TS, B), dtype=bf)
    zslab[:, :n_avail] = z[:, s_lo:s_lo + n_avail].astype(bf).transpose(2, 1, 0)
    zslab = zslab.reshape(D, TS * B)

    cslab = np.zeros((D, B, TSLOT), dtype=bf)
    cslab[:, :, :nreal] = c[:, t0:t0 + nreal].astype(bf).transpose(2, 0, 1)
    cslab = cslab.reshape(D, B * TSLOT)

    # pair-tile validity: partition p = half*64 + i, half anchored at t+half
    p_idx = np.arange(128)
    g_idx = np.arange(G)
    th = p_idx[:, None, None] // B                     # (128,1,1)
    pp = np.arange(NPAIR)[None, :, None]               # (1,7,1)
    gg = g_idx[None, None, :]                          # (1,1,31)
    slot = 2 * pp + th
    gvalid = np.where(th == 0, gg <= H - 1, (gg >= 1) & (gg <= H))
    vm = ((slot < nreal) & gvalid).astype(np.float32).reshape(128, NPAIR * G)

    # (e) masks: cm[p, g*64+j] = gvalid * (j == p%64); cmb adds the last
    # pair's slot validity (pairs 0..5 are always fully populated)
    eye = (np.arange(B)[None, :] == (p_idx % B)[:, None])          # (128,64)
    gv2 = np.where((p_idx // B)[:, None] == 0,
                   g_idx[None, :] <= H - 1,
                   (g_idx[None, :] >= 1) & (g_idx[None, :] <= H))  # (128,31)
    cma = (gv2[:, :, None] & eye[:, None, :]).astype(bf).reshape(128, G * B)
    slot6 = 12 + (p_idx // B)                                      # (128,)
    cmb = ((gv2 & (slot6 < nreal)[:, None])[:, :, None]
           & eye[:, None, :]).astype(bf).reshape(128, G * B)

    return {
        "z_bf": zslab,
        "c_bf": cslab,
        "w_bf": W.astype(bf),
        "b_f": b.astype(np.float32),
        "vm": vm,
        "cma": cma,
        "cmb": cmb,
    }


def kernel(z_seq, c_seq, W_cpc, b_cpc):
    z = np.asarray(z_seq, dtype=np.float32)
    c = np.asarray(c_seq, dtype=np.float32)
    W = np.asarray(W_cpc, dtype=np.float32)
    b = np.asarray(b_cpc, dtype=np.float32)

    nc = get_program()
    in_maps = [make_core_inputs(m, z, c, W, b) for m in range(NCORE)]

    from concourse.bass_utils import run_bass_kernel_spmd
    res = run_bass_kernel_spmd(nc, in_maps, core_ids=list(range(NCORE)))

    tot = sum(float(r["partial"].astype(np.float64).sum()) for r in res.results)
    return np.float32(tot / (TM * H * B))


if __name__ == "__main__":
    rng = np.random.default_rng(0)
    out = kernel(
        rng.standard_normal((B, T, D), dtype=np.float32),
        rng.standard_normal((B, T, D), dtype=np.float32),
        (rng.standard_normal((D, D)) / np.sqrt(D)).astype(np.float32),
        (rng.standard_normal(D) * 0.01).astype(np.float32),
    )
    print("loss:", out)


# revision 49
# speedup vs baseline: 1.4819x; 1.0302x over previous
"""CPC (contrastive predictive coding) loss on 8 Trainium2 NeuronCores.

Problem: loss = mean over (t, k, i) of cross_entropy(scores[t,k,i,:], i) with
scores[t,k,i,j] = <c_proj[i,t], z[j,t+k]> / TEMP,  c_proj = c_seq @ W + b,
t in [0, Tm), k in [1, H], i,j in [0, B).

Distribution: sequence-parallel over anchor time t.  Every core runs an
identical program over TSLOT=14 anchor slots (7 "pair tiles" of 2 consecutive
anchors each); cores with fewer real anchors carry zero-padded slots whose
contributions are removed by per-core validity masks.  Each core returns a
(128,1) vector of partial sums; the host adds them up and divides by the term
count.

Math: per valid (t,k,i) group the loss term is
    lse - pos/T = max/T + ln(S) - pos/T,   S = sum_j exp((x_j - max)/T)
so the kernel accumulates three masked sums: sum vm*ln(S) (DVE stt after a
batched Ln), sum vm*max/T (tiny per-pair stt on the negated group maxima) and
-1/T * sum cm*x (cm = validity AND j==i "eye" mask, applied to the raw PSUM
scores tile) -- the positive logits come straight out of the big scores tile,
so no separate positive-pair matmuls are needed.

Per-core device pipeline (all matmuls bf16 inputs, fp32 accumulation):
  1. One merged DMA per input tensor (all 4 contraction chunks in one
     transfer -- per-DMA fixed overhead dominates on this runtime).
  2. c_projT = (W-chunk as lhsT) @ c^T via PE into the shared PSUM pool;
     bias applied during the transposing PSUM->SBUF bf16 epilogue (ACT+DVE).
  3. Per pair tile (anchors t,t+1): scores as two 2-bank PSUM half-tiles
     (16/15 shift groups), 8 matmuls each; four half-tiles in flight.
  4. Softmax statistics, engine-constrained (only DVE/ACT read PSUM; Pool
     only runs plain tensor_tensor ALU ops):
       - ACT: one bf16 Copy per half frees the PSUM slot immediately
       - DVE: grouped reduce_max (negated) per half, from the SBUF copy
       - Pool: dsb = sb + (-max) broadcast (tensor_tensor add, bf16)
       - ACT: Exp halves + Y_ACC per-group Exp w/ accum_out (group sums)
       - DVE: grouped exp-sums; cm-masked dsb accumulation whose -max part
         telescopes away the max-compensation term (sum_j cm = vm)
       - Ln batched once at the end (avoids ACT table ping-pong with Exp)
  Every cross-engine stage is software-pipelined one pair behind its
  producer (exp-sums two pairs) so no strictly in-order engine queue --
  ACT especially -- ever head-of-line-blocks on another engine's future
  value.  Masks and bias load once per launch outside the bench loop.
"""

import numpy as np
import ml_dtypes

B, T, D = 64, 128, 512
H = 30
TEMP = 0.07
NCORE = 8
TSLOT = 14            # padded anchor slots per core -> 7 pair tiles
NPAIR = TSLOT // 2
TS = TSLOT - 1 + H    # 43 z timesteps per core (slab + horizon halo)
G = H + 1             # 31 shift groups per pair tile
KCH = D // 128        # 4 contraction chunks
TM = T - H            # 98 real anchors

GA = 16               # groups in half-tile A (B gets G - GA = 15)
Y_ACC = 5             # B-half groups whose exp+sum run fused on ACT (accum)

_REAL = [13, 13, 12, 12, 12, 12, 12, 12]
_T0 = [0, 13, 26, 38, 50, 62, 74, 86]

_CACHE = {}


def _build_program(loop_n=None, variant="full", reps=1):
    import concourse.bass as bass
    import concourse.bacc as bacc
    import concourse.tile as tile
    import concourse.mybir as mybir
    from contextlib import ExitStack

    dt = mybir.dt
    AF = mybir.ActivationFunctionType
    ALU = mybir.AluOpType
    AX = mybir.AxisListType

    nc = bacc.Bacc("TRN2", debug=False, target_bir_lowering=False,
                   num_devices=NCORE)

    z_d = nc.dram_tensor("z_bf", [D, TS * B], dt.bfloat16, kind="ExternalInput").ap()
    c_d = nc.dram_tensor("c_bf", [D, B * TSLOT], dt.bfloat16, kind="ExternalInput").ap()
    w_d = nc.dram_tensor("w_bf", [D, D], dt.bfloat16, kind="ExternalInput").ap()
    b_d = nc.dram_tensor("b_f", [D], dt.float32, kind="ExternalInput").ap()
    vm_d = nc.dram_tensor("vm", [128, NPAIR * G], dt.float32, kind="ExternalInput").ap()
    cma_d = nc.dram_tensor("cma", [128, G * B], dt.bfloat16, kind="ExternalInput").ap()
    cmb_d = nc.dram_tensor("cmb", [128, G * B], dt.bfloat16, kind="ExternalInput").ap()
    out_d = nc.dram_tensor("partial", [128, 1], dt.float32, kind="ExternalOutput").ap()
    acc_d = None
    if variant == "dbg":
        acc_d = nc.dram_tensor("accdump", [128, 15], dt.float32,
                               kind="ExternalOutput").ap()
        sall_d = nc.dram_tensor("salldump", [128, NPAIR * G], dt.float32,
                                kind="ExternalOutput").ap()
        dsb_d = nc.dram_tensor("dsbdump", [128, GA * B], dt.bfloat16,
                               kind="ExternalOutput").ap()
        sba_d = nc.dram_tensor("sbadump", [128, GA * B], dt.bfloat16,
                               kind="ExternalOutput").ap()

    NROW = B * TSLOT          # 896 c rows
    GB = G * B                # 1984 columns of a pair tile
    NACC = 15                 # 0: lnS | 1..7: posA | 8..14: posB
    inv_t = 1.0 / TEMP

    with tile.TileContext(nc) as tc, ExitStack() as ctx:
        con = ctx.enter_context(tc.tile_pool(name="con", bufs=1))
        io = ctx.enter_context(tc.tile_pool(name="io", bufs=2))
        wrk = ctx.enter_context(tc.tile_pool(name="wrk", bufs=3))
        res = ctx.enter_context(tc.tile_pool(name="res", bufs=2))
        psm = ctx.enter_context(tc.tile_pool(name="psm", bufs=4, space="PSUM"))

        # ---------- per-launch constants (outside the bench loop) ----------
        b_sb = con.tile([128, KCH], dt.float32, tag="b", name="b_sb")
        nc.sync.dma_start(b_sb[:], b_d.rearrange("(c p) -> p c", p=128))
        vm_sb = con.tile([128, NPAIR * G], dt.float32, tag="vm", name="vm_sb")
        nc.sync.dma_start(vm_sb[:], vm_d)
        cma_sb = con.tile([128, GB], dt.bfloat16, tag="cma", name="cma_sb")
        nc.sync.dma_start(cma_sb[:], cma_d)
        cmb_sb = con.tile([128, GB], dt.bfloat16, tag="cmb", name="cmb_sb")
        nc.sync.dma_start(cmb_sb[:], cmb_d)

        def _body():
            # ---------------- streamed input loads ----------------
            # one DMA per tensor (all 4 contraction chunks side-by-side in
            # the free dim) -- per-DMA fixed overhead dominates on HW.
            # w + ct on the sync queue (they gate c_proj); zt on scalar.
            w_all = io.tile([128, KCH * D], dt.bfloat16, tag="w", name="w_all")
            nc.sync.dma_start(
                w_all[:].rearrange("p (k c) -> p k c", k=KCH),
                w_d.rearrange("(k p) c -> p k c", p=128))
            ct_all = io.tile([128, KCH * NROW], dt.bfloat16, tag="ct",
                             name="ct_all")
            nc.sync.dma_start(
                ct_all[:].rearrange("p (k c) -> p k c", k=KCH),
                c_d.rearrange("(k p) c -> p k c", p=128))
            zt_all = io.tile([128, KCH * B * TS], dt.bfloat16, tag="zt",
                             name="zt_all")
            nc.sync.dma_start(
                zt_all[:].rearrange("p (k c) -> p k c", k=KCH),
                z_d.rearrange("(k p) c -> p k c", p=128))
            w_sb = [w_all[:, k * D:(k + 1) * D] for k in range(KCH)]
            ct_sb = [ct_all[:, k * NROW:(k + 1) * NROW] for k in range(KCH)]
            zt_sb = [zt_all[:, k * B * TS:(k + 1) * B * TS] for k in range(KCH)]

            acc = res.tile([128, NACC], dt.float32, tag="acc", name="acc")
            nc.vector.memset(acc[:], 0.0)
            if variant == "dmaonly":
                for k in range(KCH):
                    nc.vector.tensor_reduce(acc[:, 0:1], zt_sb[k][:, 0:64],
                                            axis=AX.X, op=ALU.add)
                    nc.vector.tensor_reduce(acc[:, 1:2], ct_sb[k][:, 0:64],
                                            axis=AX.X, op=ALU.add)
                    nc.vector.tensor_reduce(acc[:, 2:3], w_sb[k][:, 0:64],
                                            axis=AX.X, op=ALU.add)
            s_all = res.tile([128, NPAIR * G], dt.float32, tag="sall", name="s_all")

            # ---------------- c_projT (bf16, (t, i) layout) ------------
            cq_sb = []
            for m in range(KCH if variant != "dmaonly" else 0):
                psc = psm.tile([128, GA * B], dt.float32, tag="ps", name="psc")
                for (n0, nn) in ((0, 512), (512, NROW - 512)):
                    for k in range(KCH):
                        nc.tensor.matmul(
                            psc[:, n0:n0 + nn],
                            w_sb[k][:, m * 128:(m + 1) * 128],
                            ct_sb[k][:, n0:n0 + nn],
                            start=(k == 0), stop=(k == KCH - 1),
                        )
                # cq memory layout is (t, i); psc arrives as (i, t), so the
                # write transposes: out is a plain (t,i) view of cq, in is
                # the (t,i)-ordered view of psc.
                cq = io.tile([128, NROW], dt.bfloat16, tag=f"cq{m}",
                             name=f"cq_sb{m}")
                cq_v = cq[:].rearrange("p (t i) -> p t i", i=B)
                psc_v = psc[:, 0:NROW].rearrange("p (i t) -> p t i", t=TSLOT)
                bias_v = b_sb[:, m:m + 1].broadcast_to((128, NROW)).rearrange(
                    "p (t i) -> p t i", i=B)
                # epilogue in two parallel halves (i 0..31 / 32..63) so the
                # PSUM slot frees in half the time (ACT + DVE; Pool cannot
                # read PSUM)
                half = B // 2
                nc.scalar.activation(cq_v[:, :, 0:half], psc_v[:, :, 0:half],
                                     AF.Identity, bias=b_sb[:, m:m + 1])
                nc.vector.scalar_tensor_tensor(
                    cq_v[:, :, half:B], psc_v[:, :, half:B], 1.0,
                    bias_v[:, :, half:B], op0=ALU.mult, op1=ALU.add)
                cq_sb.append(cq)

            # ------- 7 pair tiles, each as two half-tiles (A: groups
            # 0..GA-1, B: groups GA..30), software-pipelined ---------------
            # Only DVE and ACT can read PSUM, and Pool only supports plain
            # tensor_tensor ALU ops, so: each half is copied once to SBUF
            # (ACT, bf16) right after its matmuls -- this frees the 2-bank
            # PSUM slot almost immediately (PE keeps its max p-state) and
            # everything downstream runs from SBUF:
            #   ACT:  copyA, copyB -> expA, expB (+Y_ACC fused accum groups)
            #   Pool: subA, subB (tensor_tensor add with broadcast -max)
            #   DVE:  maxA, maxB; pos accums (telescoped, from dsb);
            #         exp-sums of pair p-1 (delayed so DVE never waits)
            GBB = G - GA          # groups in half B
            UB = GBB - Y_ACC      # B groups summed on DVE (rest via ACT accum)
            prev = None
            pend_d = []

            def stage_exp(st):
                # ACT: Exp halves + Y_ACC fused exp-sum groups (B tail)
                dsba, dsbb, cm3, s_t, p = st
                esba = wrk.tile([128, GA * B], dt.bfloat16, tag="esba",
                                name="esba")
                nc.scalar.activation(esba[:], dsba[:], AF.Exp, scale=inv_t)
                esbb = wrk.tile([128, UB * B], dt.bfloat16, tag="esbb",
                                name="esbb")
                nc.scalar.activation(esbb[:], dsbb[:, 0:UB * B], AF.Exp,
                                     scale=inv_t)
                for f in range(Y_ACC):
                    g = GA + UB + f
                    fe = wrk.tile([128, B], dt.bfloat16, tag=f"fe{f}",
                                  name=f"fe{f}")
                    nc.scalar.activation(
                        fe[:], dsbb[:, (UB + f) * B:(UB + f + 1) * B], AF.Exp,
                        scale=inv_t, accum_out=s_t[:, g:g + 1])
                return (esba, esbb, s_t)

            def stage_pos(st):
                # DVE: positive terms from dsb (the -max parts telescope
                # against the max-compensation since sum_j cm = vm)
                dsba, dsbb, cm3, s_t, p = st
                da3 = dsba[:].rearrange("p (g j) -> p g j", j=B)
                db3 = dsbb[:].rearrange("p (g j) -> p g j", j=B)
                junka = wrk.tile([128, GA * B], dt.bfloat16, tag="junka",
                                 name="junka")
                nc.vector.scalar_tensor_tensor(
                    junka[:].rearrange("p (g j) -> p g j", j=B),
                    da3, -inv_t, cm3[:, 0:GA, :],
                    op0=ALU.mult, op1=ALU.mult,
                    accum_out=acc[:, 1 + p:2 + p])
                junkb = wrk.tile([128, GBB * B], dt.bfloat16, tag="junkb",
                                 name="junkb")
                nc.vector.scalar_tensor_tensor(
                    junkb[:].rearrange("p (g j) -> p g j", j=B),
                    db3, -inv_t, cm3[:, GA:G, :],
                    op0=ALU.mult, op1=ALU.mult,
                    accum_out=acc[:, 8 + p:9 + p])

            def stage_d(st):
                # DVE: grouped exp-sums
                pesba, pesbb, pst = st
                nc.vector.tensor_reduce(
                    pst[:, 0:GA],
                    pesba[:].rearrange("p (g j) -> p g j", j=B),
                    axis=AX.X, op=ALU.add)
                nc.vector.tensor_reduce(
                    pst[:, GA:GA + UB],
                    pesbb[:].rearrange("p (g j) -> p g j", j=B),
                    axis=AX.X, op=ALU.add)
            for p in range(NPAIR if variant != "dmaonly" else 0):
                psa = psm.tile([128, GA * B], dt.float32, tag="ps", name="psa")
                for (g0, gn) in ((0, 8), (8, GA - 8)):
                    for k in range(KCH):
                        nc.tensor.matmul(
                            psa[:, g0 * B:(g0 + gn) * B],
                            cq_sb[k][:, 2 * p * B:(2 * p + 2) * B],
                            zt_sb[k][:, (2 * p + g0) * B:(2 * p + g0 + gn) * B],
                            start=(k == 0), stop=(k == KCH - 1),
                        )
                psb = psm.tile([128, GA * B], dt.float32, tag="ps", name="psb")
                for (g0, gn) in ((GA, 8), (GA + 8, GBB - 8)):
                    for k in range(KCH):
                        nc.tensor.matmul(
                            psb[:, (g0 - GA) * B:(g0 - GA + gn) * B],
                            cq_sb[k][:, 2 * p * B:(2 * p + 2) * B],
                            zt_sb[k][:, (2 * p + g0) * B:(2 * p + g0 + gn) * B],
                            start=(k == 0), stop=(k == KCH - 1),
                        )

                if variant == "noce":
                    junkc = wrk.tile([128, 1], dt.float32, tag="junkc",
                                     name="junkc")
                    nc.vector.tensor_reduce(junkc[:], psa[:, 0:B],
                                            axis=AX.X, op=ALU.add)
                    nc.vector.tensor_reduce(junkc[:], psb[:, 0:B],
                                            axis=AX.X, op=ALU.add)
                    continue

                cm_sb = cmb_sb if p == NPAIR - 1 else cma_sb
                cm3 = cm_sb[:].rearrange("p (g j) -> p g j", j=B)
                s_t = s_all[:, p * G:(p + 1) * G]

                # ACT: one bf16 copy per half frees the PSUM slot.  These
                # lead ACT's (strictly in-order) stream each pair; the exps
                # of the previous pair are emitted after them so a copy is
                # never queued behind work that waits on Pool/DVE.
                sba = wrk.tile([128, GA * B], dt.bfloat16, tag="sba", name="sba")
                nc.scalar.activation(sba[:], psa[:], AF.Copy)
                sbb = wrk.tile([128, GBB * B], dt.bfloat16, tag="sbb", name="sbb")
                nc.scalar.activation(sbb[:], psb[:, 0:GBB * B], AF.Copy)
                sa3 = sba[:].rearrange("p (g j) -> p g j", j=B)
                sb3 = sbb[:].rearrange("p (g j) -> p g j", j=B)

                # DVE: grouped max halves (negated, separate tiles)
                nma = wrk.tile([128, GA], dt.float32, tag="nma", name="nma")
                nc.vector.tensor_reduce(nma[:], sa3, axis=AX.X,
                                        op=ALU.max, negate=True)
                nmb = wrk.tile([128, GBB], dt.float32, tag="nmb", name="nmb")
                nc.vector.tensor_reduce(nmb[:], sb3, axis=AX.X,
                                        op=ALU.max, negate=True)

                # Pool: dsb = sb + (-max) broadcast (bf16), per half
                dsba = wrk.tile([128, GA * B], dt.bfloat16, tag="dsba",
                                name="dsba")
                nc.gpsimd.tensor_tensor(
                    dsba[:].rearrange("p (g j) -> p g j", j=B),
                    sa3, nma[:].broadcast_to((128, GA, B)), op=ALU.add)
                dsbb = wrk.tile([128, GBB * B], dt.bfloat16, tag="dsbb",
                                name="dsbb")
                nc.gpsimd.tensor_tensor(
                    dsbb[:].rearrange("p (g j) -> p g j", j=B),
                    sb3, nmb[:].broadcast_to((128, GBB, B)), op=ALU.add)

                if variant == "dbg" and p == 0:
                    nc.gpsimd.dma_start(dsb_d, dsba[:])
                    nc.gpsimd.dma_start(sba_d, sba[:])

                # software-pipelined tails: exps + pos for pair p-1,
                # exp-sums for pair p-2
                if prev is not None:
                    pend_d.append(stage_exp(prev))
                    stage_pos(prev)
                if len(pend_d) > 1:
                    stage_d(pend_d.pop(0))
                prev = (dsba, dsbb, cm3, s_t, p)
            if variant in ("full", "dbg") and prev is not None:
                pend_d.append(stage_exp(prev))
                stage_pos(prev)
                for st in pend_d:
                    stage_d(st)

            if variant == "dbg":
                nc.gpsimd.dma_start(acc_d, acc[:])
                nc.gpsimd.dma_start(sall_d, s_all[:])
            if variant not in ("full", "dbg"):
                part0 = res.tile([128, 1], dt.float32, tag="part", name="part0")
                nc.vector.tensor_reduce(part0[:], acc[:], axis=AX.X, op=ALU.add)
                nc.gpsimd.dma_start(out_d, part0[:])
                return
            logs_all = res.tile([128, NPAIR * G], dt.float32, tag="logsall",
                                name="logs_all")
            nc.scalar.activation(logs_all[:], s_all[:], AF.Ln)
            junkl = res.tile([128, NPAIR * G], dt.float32, tag="junkl", name="junkl")
            nc.vector.scalar_tensor_tensor(
                junkl[:], logs_all[:], 1.0, vm_sb[:], op0=ALU.mult, op1=ALU.mult,
                accum_out=acc[:, 0:1])
            part = res.tile([128, 1], dt.float32, tag="part", name="part")
            nc.vector.tensor_reduce(part[:], acc[:], axis=AX.X, op=ALU.add)
            nc.gpsimd.dma_start(out_d, part[:])

        if loop_n:
            with tc.For_i(0, loop_n, 1):
                for _ in range(reps):
                    _body()
        else:
            for _ in range(reps):
                _body()

    nc.compile()
    return nc


def get_program(loop_n=None, variant="full", reps=1):
    key = ("nc", loop_n, variant, reps)
    if key not in _CACHE:
        _CACHE[key] = _build_program(loop_n, variant, reps)
    return _CACHE[key]


def make_core_inputs(m, z, c, W, b):
    """Host-side sharding + bf16 cast for core m."""
    bf = ml_dtypes.bfloat16
    t0, nreal = _T0[m], _REAL[m]

    # device-side layouts: zT (D, (s, i)), cT (D, (i, t)) -- transposed on
    # the host so the device does plain contiguous DMA loads (no xbar)
    s_lo = t0 + 1
    n_avail = min(TS, T - s_lo)
    zslab = np.zeros((D, TS, B), dtype=bf)
    zslab[:, :n_avail] = z[:, s_lo:s_lo + n_avail].astype(bf).transpose(2, 1, 0)
    zslab = zslab.reshape(D, TS * B)

    cslab = np.zeros((D, B, TSLOT), dtype=bf)
    cslab[:, :, :nreal] = c[:, t0:t0 + nreal].astype(bf).transpose(2, 0, 1)
    cslab = cslab.reshape(D, B * TSLOT)

    # pair-tile validity: partition p = half*64 + i, half anchored at t+half
    p_idx = np.arange(128)
    g_idx = np.arange(G)
    th = p_idx[:, None, None] // B                     # (128,1,1)
    pp = np.arange(NPAIR)[None, :, None]               # (1,7,1)
    gg = g_idx[None, None, :]                          # (1,1,31)
    slot = 2 * pp + th
    gvalid = np.where(th == 0, gg <= H - 1, (gg >= 1) & (gg <= H))
    vm = ((slot < nreal) & gvalid).astype(np.float32).reshape(128, NPAIR * G)

    # (e) masks: cm[p, g*64+j] = gvalid * (j == p%64); cmb adds the last
    # pair's slot validity (pairs 0..5 are always fully populated)
    eye = (np.arange(B)[None, :] == (p_idx % B)[:, None])          # (128,64)
    gv2 = np.where((p_idx // B)[:, None] == 0,
                   g_idx[None, :] <= H - 1,
                   (g_idx[None, :] >= 1) & (g_idx[None, :] <= H))  # (128,31)
    cma = (gv2[:, :, None] & eye[:, None, :]).astype(bf).reshape(128, G * B)
    slot6 = 12 + (p_idx // B)                                      # (128,)
    cmb = ((gv2 & (slot6 < nreal)[:, None])[:, :, None]
           & eye[:, None, :]).astype(bf).reshape(128, G * B)

    return {
        "z_bf": zslab,
        "c_bf": cslab,
        "w_bf": W.astype(bf),
        "b_f": b.astype(np.float32),
        "vm": vm,
        "cma": cma,
        "cmb": cmb,
    }


def kernel(z_seq, c_seq, W_cpc, b_cpc):
    z = np.asarray(z_seq, dtype=np.float32)
    c = np.asarray(c_seq, dtype=np.float32)
    W = np.asarray(W_cpc, dtype=np.float32)
    b = np.asarray(b_cpc, dtype=np.float32)

    nc = get_program()
    in_maps = [make_core_inputs(m, z, c, W, b) for m in range(NCORE)]

    from concourse.bass_utils import run_bass_kernel_spmd
    res = run_bass_kernel_spmd(nc, in_maps, core_ids=list(range(NCORE)))

    tot = sum(float(r["partial"].astype(np.float64).sum()) for r in res.results)
    return np.float32(tot / (TM * H * B))


if __name__ == "__main__":
    rng = np.random.default_rng(0)
    out = kernel(
        rng.standard_normal((B, T, D), dtype=np.float32),
        rng.standard_normal((B, T, D), dtype=np.float32),
        (rng.standard_normal((D, D)) / np.sqrt(D)).astype(np.float32),
        (rng.standard_normal(D) * 0.01).astype(np.float32),
    )
    print("loss:", out)
